# revision 50
# baseline (speedup 1.0000x reference)
"""Self-contained MaxK-GIN conv kernel for 8 trn2 NeuronCores."""
import numpy as np

# ---- walrus compat patches (single sync-wait per instruction) ----
"""Compat patches for this container's walrus: it accepts at most ONE sync-wait
per instruction. Fix up the final BIR by hoisting extra waits onto injected
nops placed immediately before the instruction on the same engine (engines are
in-order, so waiting earlier is semantically identical)."""
import concourse.bass as bass
import concourse.mybir as mybir

_nop_ctr = [0]

def _split_multi_waits(m):
    for f in m.functions:
        for b in f.blocks:
            insts = b.instructions
            out = []
            changed = False
            for inst in insts:
                si = inst.sync_info
                if si is not None and len(si.on_wait) > 1:
                    waits = list(si.on_wait)
                    for w in waits[:-1]:
                        _nop_ctr[0] += 1
                        nop = mybir.InstNoOp(name=f"waitnop-{_nop_ctr[0]}", ins=[], outs=[])
                        nop.engine = inst.engine
                        nop.sync_info = mybir.SyncInfo(on_wait=[w], on_update=[])
                        out.append(nop)
                    inst.sync_info = mybir.SyncInfo(
                        on_wait=[waits[-1]], on_update=list(si.on_update))
                    changed = True
                out.append(inst)
            if changed:
                b.instructions = out

_orig_to_json_bytes = bass.Bass.to_json_bytes

def _patched_to_json_bytes(self):
    if not getattr(self, "_isa_subclasses_lowered", False):
        mybir.codegen_inst_isa_subclasses(self)
        self._isa_subclasses_lowered = True
    _split_multi_waits(self.m)
    return _orig_to_json_bytes(self)

bass.Bass.to_json_bytes = _patched_to_json_bytes


# ---- kernel library ----
"""MaxK-GIN conv kernel v3 for trn2, 8-core SPMD — bf16 data path.

Structure: phase 1 sparsifies the local feat shard (top-32/row via DVE
max+match_replace), phase 2 AllGathers the bf16 sparse table, phase 3
gathers per-edge source rows (SWDGE dma_gather over 4 queues — the
bottleneck resource: ~27GB/s per queue) and scatter-adds them into
per-dst-block PSUM via selection matmuls, then applies the GIN residual
and 2-layer MLP.

v3 tuning (hardware-measured, ~404us/call vs 654us for v2):
- W=128 ladder windows: fewest selection groups (813 vs 929), fewer
  gather descriptors — descriptor count is what gathers pay for.
- 10 rotating message-tile slots keep all 4 SWDGE queues draining.
- sel matrices + bf16 feat residual streamed per block (SBUF headroom);
  idx resident.
"""
import numpy as np
import ml_dtypes

import concourse.bass as bass
import concourse.mybir as mybir
import concourse.tile as tile
from concourse import library_config

F32 = mybir.dt.float32
BF16 = mybir.dt.bfloat16
I16 = mybir.dt.int16
I32 = mybir.dt.int32

D = 128
MAXK = 32
W = 128         # selection window width (columns)
BLK = 512       # dst columns per PSUM block
NEG = -3.0e38   # match_replace fill
BF = ml_dtypes.bfloat16


# ---------------------------------------------------------------- host prep
def host_prep(feat, W1, b1, W2, b2, eps, edge_src, edge_dst, M,
              loc_split=False):
    N = feat.shape[0]
    NS = N // M
    NB = (NS + BLK - 1) // BLK
    HALF = (N // 2 + 127) & ~127  # table split point for int16 gather indices
    assert HALF <= 32768 and N - HALF <= 32768

    feat = np.asarray(feat, np.float32)
    edge_src = np.asarray(edge_src, np.int64)
    edge_dst = np.asarray(edge_dst, np.int64)
    eps_v = float(np.asarray(eps).reshape(-1)[0])

    per_core = []
    for c in range(M):
        lo = c * NS
        m = (edge_dst >= lo) & (edge_dst < lo + NS)
        ed = edge_dst[m] - lo
        es = edge_src[m]
        order = np.argsort(ed, kind="stable")
        per_core.append((ed[order], es[order]))

    def ladder_offsets(span, g):
        if g <= 0:
            return []
        if g == 1:
            return [0]
        lim = max(0, min(span, BLK) - W)
        step = lim / (g - 1) if g > 1 else 0
        return [min(int(round(j * step)), lim) for j in range(g)]

    def ladder_pack(cols, offs):
        g = len(offs)
        cnt = [0] * g
        j = 0
        for c in cols:
            while j < g and (c >= offs[j] + W or cnt[j] >= 128):
                j += 1
            if j >= g or c < offs[j]:
                return None
            cnt[j] += 1
        return cnt

    def min_groups(cols, span):
        if len(cols) == 0:
            return 0
        g = max(1, (len(cols) + 127) // 128)
        while ladder_pack(cols, ladder_offsets(span, g)) is None:
            g += 1
        return g

    NH = 3  # categories per block: 0=local shard, 1=remote-lo, 2=remote-hi
    groups = [[[None] * NH for _ in range(NB)] for _ in range(M)]
    G = np.zeros((NB, NH), np.int64)
    for c in range(M):
        ed, es = per_core[c]
        slo, shi = c * NS, (c + 1) * NS
        for b in range(NB):
            blo = b * BLK
            bw = min(BLK, NS - blo)
            bm = (ed >= blo) & (ed < blo + bw)
            bcols = ed[bm] - blo
            bsrc = es[bm]
            loc = ((bsrc >= slo) & (bsrc < shi) if loc_split
                   else np.zeros(bsrc.shape, bool))
            for h in range(NH):
                if h == 0:
                    hm = loc
                elif h == 1:
                    hm = (bsrc < HALF) & ~loc
                else:
                    hm = (bsrc >= HALF) & ~loc
                hc = bsrc[hm]
                hcols = bcols[hm]
                o = np.argsort(hcols, kind="stable")
                groups[c][b][h] = (hcols[o], hc[o])
                G[b, h] = max(G[b, h], min_groups(np.sort(hcols), bw))

    for b in range(NB):
        bw = min(BLK, NS - b * BLK)
        for h in range(NH):
            ok = False
            while not ok:
                ok = True
                offs = ladder_offsets(bw, int(G[b, h]))
                for c in range(M):
                    if ladder_pack(np.sort(groups[c][b][h][0]), offs) is None:
                        G[b, h] += 1
                        ok = False
                        break

    G_tot = int(G.sum())
    blk_cols = [(int(G[b].sum()) * 128) // 16 for b in range(NB)]
    tot_cols = sum(blk_cols)

    in_maps = []
    for c in range(M):
        idx_all = np.zeros((128, tot_cols), np.int16)
        sel_all = np.zeros((128, G_tot * W), BF)
        col0 = 0
        gflat = 0
        for b in range(NB):
            bw = min(BLK, NS - b * BLK)
            flat_idx = np.zeros((int(G[b].sum()) * 128,), np.int16)
            gi = 0
            for h in range(NH):
                gh = int(G[b, h])
                offs = ladder_offsets(bw, gh)
                hcols, hsrcs = groups[c][b][h]
                cnt = ladder_pack(hcols, offs)
                assert cnt is not None
                base = c * NS if h == 0 else (0 if h == 1 else HALF)
                p = 0
                for k in range(gh):
                    nmsg = cnt[k]
                    gcols = hcols[p:p + nmsg]
                    gsrcs = hsrcs[p:p + nmsg]
                    p += nmsg
                    flat_idx[gi * 128: gi * 128 + nmsg] = (gsrcs - base).astype(np.int16)
                    off = offs[k]
                    rel = gcols - off
                    assert nmsg == 0 or ((rel >= 0).all() and (rel < W).all())
                    sel_all[np.arange(nmsg), gflat * W + rel] = BF(1.0)
                    gi += 1
                    gflat += 1
                assert p == len(hcols)
            wrapped = flat_idx.reshape(-1, 16).T
            nb_cols = blk_cols[b]
            for r in range(8):
                idx_all[16 * r:16 * (r + 1), col0:col0 + nb_cols] = wrapped
            col0 += nb_cols
        in_maps.append({
            "feat_shard": feat[c * NS:(c + 1) * NS],
            "idx_all": idx_all,
            "sel_all": sel_all,
            "w1t": np.ascontiguousarray(np.asarray(W1, np.float32).T).astype(BF),
            "b1c": np.asarray(b1, np.float32).reshape(128, 1),
            "w2t": np.ascontiguousarray(np.asarray(W2, np.float32).T).astype(BF),
            "b2rep": np.tile(np.asarray(b2, np.float32), (128, 4)).reshape(128, 512),
            "ident_s": (np.eye(128, dtype=np.float32) * (1.0 + eps_v)).astype(BF),
        })

    lad = []
    for b in range(NB):
        bw = min(BLK, NS - b * BLK)
        lad.append([ladder_offsets(bw, int(G[b, h])) for h in range(NH)])
    meta = dict(N=N, M=M, NS=NS, NB=NB, HALF=HALF,
                G=G, G_tot=G_tot, blk_cols=blk_cols, tot_cols=tot_cols, lad=lad)
    return meta, in_maps


# ---------------------------------------------------------------- device build
def build_nc(meta, phases=(1, 2, 3), reps=1, no_selmm=False, no_gather=False,
             msg_bufs=1, gather_cs=None, ntags=4, nqueues=4, fat=False,
             scratch=16384):
    """gather_cs: max groups per dma_gather (None = no split); ntags: number
    of rotating message-tile slots (in-flight gathers)."""
    phases = set(phases)
    N, M, NS, NB = meta["N"], meta["M"], meta["NS"], meta["NB"]
    HALF, G, G_tot = meta["HALF"], meta["G"], meta["G_tot"]
    blk_cols, tot_cols = meta["blk_cols"], meta["tot_cols"]
    Gmax_half = int(G.max())
    NT = (NS + 127) // 128          # 128-row tiles in the shard

    nc = bass.Bass(num_swdge_queues=4, dynamic_dma_scratch_size=scratch)
    feat_shard = nc.dram_tensor("feat_shard", [NS, D], F32, kind="ExternalInput")
    idx_all = nc.dram_tensor("idx_all", [128, tot_cols], I16, kind="ExternalInput")
    sel_all = nc.dram_tensor("sel_all", [128, G_tot * W], BF16, kind="ExternalInput")
    w1t = nc.dram_tensor("w1t", [D, D], BF16, kind="ExternalInput")
    b1c = nc.dram_tensor("b1c", [D, 1], F32, kind="ExternalInput")
    w2t = nc.dram_tensor("w2t", [D, D], BF16, kind="ExternalInput")
    b2rep = nc.dram_tensor("b2rep", [D, BLK], F32, kind="ExternalInput")
    ident_s = nc.dram_tensor("ident_s", [D, D], BF16, kind="ExternalInput")
    out = nc.dram_tensor("out", [NS, D], F32, kind="ExternalOutput")

    shard_fs = nc.dram_tensor("shard_fs", [NS, D], BF16)
    shard_fbf = nc.dram_tensor("shard_fbf", [NS, D], BF16)
    table = nc.dram_tensor("table", [N + (128 if fat else 0), D], BF16,
                           addr_space="Shared")

    nc.gpsimd.load_library(library_config.mlp)

    reg_cache = {}

    def nreg(v):
        if v not in reg_cache:
            reg_cache[v] = nc.gpsimd.to_reg(v)
        return reg_cache[v]

    # phase-1 chunks: 4 tiles (512 rows) per DMA, last chunk is the remainder
    chunks = []
    r = 0
    while r < NS:
        nrow = min(512, NS - r)
        chunks.append((r, nrow))
        r += nrow

    with tile.TileContext(nc) as tc:
      for _rep in range(reps):
        with (
            tc.tile_pool(name=f"const{_rep}", bufs=1) as constp,
            tc.tile_pool(name=f"keep{_rep}", bufs=1) as keepp,
        ):
            # ---------------- persistent SBUF state
            w1t_sb = constp.tile([D, D], BF16)
            nc.sync.dma_start(w1t_sb[:], w1t[:, :])
            w2t_sb = constp.tile([D, D], BF16)
            nc.sync.dma_start(w2t_sb[:], w2t[:, :])
            b1_sb = constp.tile([D, 1], F32)
            nc.sync.dma_start(b1_sb[:], b1c[:, :])
            b2_sb = constp.tile([D, BLK], F32)
            nc.sync.dma_start(b2_sb[:], b2rep[:, :])
            id_sb = constp.tile([D, D], BF16)
            nc.sync.dma_start(id_sb[:], ident_s[:, :])
            zrow = constp.tile([1, D], BF16)
            nc.vector.memset(zrow[:], 0.0)
            orow = constp.tile([1, BLK], BF16)
            nc.vector.memset(orow[:], 1.0)
            if no_gather:
                dummy_src = constp.tile([128, D], BF16)
                nc.vector.memset(dummy_src[:], 0.5)

            idx_sb = keepp.tile([128, tot_cols], I16)
            nc.sync.dma_start(idx_sb[:], idx_all[:, :])

            # ---------------- phase 1: top-32 threshold + sparsify (bf16 out)
            with (
                tc.tile_pool(name=f"p1_{_rep}", bufs=3) as p1p,
                tc.tile_pool(name=f"p1w{_rep}", bufs=2) as p1w,
            ):
                for ci, (r0, nrow) in (enumerate(chunks) if 1 in phases else []):
                    nt = (nrow + 127) // 128
                    ft = p1p.tile([128, 4, D], F32, tag="ft")
                    if _rep > 0 and ci == 0:
                        # cross-rep serializer for replicated timing builds:
                        # read prev rep's output into ft, then overwrite
                        nc.sync.dma_start(ft[:, 0, :], out[0:128, :])
                    if nrow % 128 == 0:
                        nc.sync.dma_start(
                            ft[:, 0:nt, :],
                            feat_shard.ap()[r0:r0 + nrow, :]
                            .rearrange("(t p) c -> p t c", p=128))
                    else:
                        assert nt == 1
                        nc.sync.dma_start(ft[:nrow, 0, :],
                                          feat_shard[r0:r0 + nrow, :])
                    fs = p1p.tile([128, 4, D], BF16, tag="fs")
                    fbf = p1p.tile([128, 4, D], BF16, tag="fbf")
                    for j in range(nt):
                        p = min(128, nrow - j * 128)
                        m8 = p1w.tile([128, 8], F32, tag="m8")
                        wa = p1w.tile([128, D], F32, tag="wa")
                        wb = p1w.tile([128, D], F32, tag="wb")
                        nc.vector.max(m8[:p], ft[:p, j, :])
                        nc.vector.match_replace(wa[:p, :], m8[:p], ft[:p, j, :], NEG)
                        nc.vector.max(m8[:p], wa[:p, :])
                        nc.vector.match_replace(wb[:p, :], m8[:p], wa[:p, :], NEG)
                        nc.vector.max(m8[:p], wb[:p, :])
                        nc.vector.match_replace(wa[:p, :], m8[:p], wb[:p, :], NEG)
                        nc.vector.max(m8[:p], wa[:p, :])
                        mask = p1w.tile([128, D], F32, tag="mask")
                        nc.vector.tensor_scalar(mask[:p, :], ft[:p, j, :],
                                                m8[:p, 7:8], None,
                                                op0=mybir.AluOpType.is_ge)
                        nc.vector.tensor_mul(fs[:p, j, :], ft[:p, j, :], mask[:p, :])
                        # bf16 feat copy for the residual matmuls (via DRAM)
                        nc.scalar.copy(fbf[:p, j, :], ft[:p, j, :])
                    if nrow % 128 == 0:
                        nc.sync.dma_start(
                            shard_fs.ap()[r0:r0 + nrow, :]
                            .rearrange("(t p) c -> p t c", p=128),
                            fs[:, 0:nt, :])
                        nc.sync.dma_start(
                            shard_fbf.ap()[r0:r0 + nrow, :]
                            .rearrange("(t p) c -> p t c", p=128),
                            fbf[:, 0:nt, :])
                    else:
                        nc.sync.dma_start(shard_fs[r0:r0 + nrow, :],
                                          fs[:nrow, 0, :])
                        nc.sync.dma_start(shard_fbf[r0:r0 + nrow, :],
                                          fbf[:nrow, 0, :])

            # ---------------- phase 3: dst blocks (local gathers precede the
            # allgather on Pool so their drains overlap the collective)
            Gmax_blk = max(int(G[b].sum()) for b in range(NB))
            with (
                tc.tile_pool(name=f"msg{_rep}", bufs=msg_bufs) as msgp,
                tc.tile_pool(name=f"loc{_rep}", bufs=1) as locp,
                tc.tile_pool(name=f"mlp{_rep}", bufs=2) as mlpp,
                tc.tile_pool(name=f"sel{_rep}", bufs=2) as selp,
                tc.tile_pool(name=f"ps{_rep}", bufs=2, space="PSUM") as psp,
            ):
                qi = 0
                loc_tiles = {}
                if 3 in phases:
                    col0 = 0
                    for b in range(NB):
                        gloc = int(G[b, 0])
                        if not gloc:
                            col0 += blk_cols[b]
                            continue
                        lt = locp.tile([128, gloc, D], BF16,
                                       tag=f"loc{b}")
                        if not no_gather:
                            nc.gpsimd.dma_gather(
                                out_ap=lt[:, 0:gloc, :],
                                in_ap=shard_fs[0:NS, :],
                                idxs_ap=idx_sb[:, col0:col0 + gloc * 8],
                                num_idxs=gloc * 128,
                                num_idxs_reg=nreg(gloc * 128),
                                elem_size=D, single_packet=False,
                                queue_num=qi % nqueues)
                            qi += 1
                        loc_tiles[b] = lt
                        col0 += blk_cols[b]

                # ---------------- phase 2: allgather (bf16)
                if 2 in phases:
                    nc.gpsimd.collective_compute(
                        "AllGather", mybir.AluOpType.bypass,
                        replica_groups=[list(range(M))],
                        ins=[shard_fs.ap().opt()],
                        outs=[table[0:N, :].opt()],
                    )

                col0 = 0
                gflat = 0
                for b in (range(NB) if 3 in phases else []):
                    bw = min(BLK, NS - b * BLK)
                    gloc = int(G[b, 0])
                    glo, ghi = int(G[b, 1]), int(G[b, 2])
                    gb = gloc + glo + ghi
                    nb_cols = blk_cols[b]

                    # streamed per-block sel slice + bf16 feat slice
                    selb = selp.tile([128, Gmax_blk * W], BF16, tag="sel")
                    nc.sync.dma_start(
                        selb[:, 0:gb * W],
                        sel_all[:, gflat * W:(gflat + gb) * W])
                    fb = selp.tile([128, 4, D], BF16, tag="fb")
                    if bw % 128 == 0:
                        nc.sync.dma_start(
                            fb[:, 0:bw // 128, :],
                            shard_fbf.ap()[b * BLK:b * BLK + bw, :]
                            .rearrange("(t p) c -> p t c", p=128))
                    else:
                        nc.sync.dma_start(fb[:bw, 0, :],
                                          shard_fbf[b * BLK:b * BLK + bw, :])

                    CS = gather_cs if gather_cs else Gmax_half
                    TS = min(CS, Gmax_half)   # tile size in groups

                    def gather_half(nh, icol0, tlo, thi):
                        """Gather nh groups for one table half into chunked
                        tiles; returns list of (tile, base_group)."""
                        nonlocal qi
                        tiles = []
                        ew = 2 * D if fat else D
                        for o in range(0, nh, CS):
                            n = min(CS, nh - o)
                            md = msgp.tile([128, TS, ew], BF16,
                                           tag=f"mq{qi % ntags}")
                            if not no_gather:
                                nc.gpsimd.dma_gather(
                                    out_ap=md[:, 0:n, :],
                                    in_ap=table[tlo:thi + (128 if fat else 0), :],
                                    idxs_ap=idx_sb[:, icol0 + o * 8:
                                                   icol0 + (o + n) * 8],
                                    num_idxs=n * 128,
                                    num_idxs_reg=nreg(n * 128),
                                    elem_size=ew,
                                    elem_step=(D if fat else None),
                                    single_packet=False,
                                    queue_num=qi % nqueues)
                            qi += 1
                            tiles.append(md)
                        return tiles

                    lo_tiles = gather_half(glo, col0 + gloc * 8, 0, HALF)
                    hi_tiles = gather_half(ghi, col0 + (gloc + glo) * 8,
                                           HALF, N)

                    def src_of(g):
                        if g < gloc:
                            return loc_tiles[b][:, g, :]
                        if g < gloc + glo:
                            gl = g - gloc
                            return lo_tiles[gl // CS][:, gl % CS, 0:D]
                        gh = g - gloc - glo
                        return hi_tiles[gh // CS][:, gh % CS, 0:D]

                    hps = psp.tile([D, BLK], F32, tag="hps")
                    nc.tensor.matmul(hps[:, :], zrow[:, :], orow[:, :],
                                     start=True, stop=False)
                    if no_selmm and not no_gather:
                        # consumers so gather completion is still awaited
                        mr = mlpp.tile([128, 8], F32, tag="mr")
                        for ti, md in enumerate(lo_tiles + hi_tiles):
                            nc.vector.max(mr[:], md[:, :, 0:1])
                        if gloc:
                            nc.vector.max(mr[:], loc_tiles[b][:, :, 0:1])
                    lad = meta["lad"][b]
                    loffs = list(lad[0]) + list(lad[1]) + list(lad[2])
                    for g in (range(gb) if not no_selmm else []):
                        off = loffs[g]
                        if no_gather:
                            src = dummy_src[:, :]
                        else:
                            src = src_of(g)
                        nc.tensor.matmul(
                            hps[:, off:off + W],
                            src,
                            selb[:, g * W:(g + 1) * W],
                            start=False, stop=False)
                    # residual: += (1+eps) * feat^T from streamed bf16 copy
                    rr, t = 0, 0
                    while rr < bw:
                        p = min(128, bw - rr)
                        nc.tensor.matmul(
                            hps[:, rr:rr + p],
                            fb[:p, t, :],
                            id_sb[:p, :p],
                            start=False, stop=(rr + p >= bw))
                        rr += p
                        t += 1

                    # MLP layer 1 (h cast to bf16 on ACT)
                    h_sb = mlpp.tile([D, BLK], BF16, tag="h")
                    nc.scalar.copy(h_sb[:, :bw], hps[:, :bw])
                    y1ps = psp.tile([D, BLK], F32, tag="y1ps")
                    nc.tensor.matmul(y1ps[:, :bw], w1t_sb[:, :], h_sb[:, :bw],
                                     start=True, stop=True)
                    y1_sb = mlpp.tile([D, BLK], BF16, tag="y1")
                    nc.scalar.activation(y1_sb[:, :bw], y1ps[:, :bw],
                                         mybir.ActivationFunctionType.Relu,
                                         bias=b1_sb[:, 0:1], scale=1.0)
                    # MLP layer 2 fused with output transpose
                    y2ps = psp.tile([128, 4, D], F32, tag="y2ps")
                    o_sb = mlpp.tile([128, 4, D], F32, tag="o")
                    rr, t = 0, 0
                    while rr < bw:
                        p = min(128, bw - rr)
                        nc.tensor.matmul(
                            y2ps[:p, t, :],
                            y1_sb[:, rr:rr + p],
                            w2t_sb[:, :],
                            start=True, stop=True)
                        nc.vector.tensor_add(o_sb[:p, t, :], y2ps[:p, t, :],
                                             b2_sb[:p, t * D:(t + 1) * D])
                        rr += p
                        t += 1
                    if bw % 128 == 0:
                        nc.sync.dma_start(
                            out.ap()[b * BLK:b * BLK + bw, :]
                            .rearrange("(t p) c -> p t c", p=128),
                            o_sb[:, 0:t, :])
                    else:
                        assert t == 1
                        nc.sync.dma_start(out[b * BLK:b * BLK + bw, :],
                                          o_sb[:bw, 0, :])

                    col0 += nb_cols
                    gflat += gb
    return nc


# ---------------------------------------------------------------- entry point
def kernel(**inputs):
    from concourse.bass_utils import run_bass_kernel_spmd

    M = 8
    feat = np.asarray(inputs["feat"], np.float32)
    meta, in_maps = host_prep(
        feat, inputs["W1"], inputs["b1"], inputs["W2"], inputs["b2"],
        inputs["eps"], inputs["edge_src"], inputs["edge_dst"], M)
    nc = build_nc(meta, ntags=10)
    res = run_bass_kernel_spmd(nc, in_maps, core_ids=list(range(M)))
    out = np.concatenate([res.results[c]["out"] for c in range(M)], axis=0)
    return out.astype(np.float32)



# revision 53
# speedup vs baseline: 1.2720x; 1.2720x over previous
"""Self-contained MaxK-GIN conv kernel for 8 trn2 NeuronCores."""
import numpy as np

# ---- walrus compat patches (single sync-wait per instruction) ----
"""Compat patches for this container's walrus: it accepts at most ONE sync-wait
per instruction. Fix up the final BIR by hoisting extra waits onto injected
nops placed immediately before the instruction on the same engine (engines are
in-order, so waiting earlier is semantically identical)."""
import concourse.bass as bass
import concourse.mybir as mybir

_nop_ctr = [0]

def _split_multi_waits(m):
    for f in m.functions:
        for b in f.blocks:
            insts = b.instructions
            out = []
            changed = False
            for inst in insts:
                si = inst.sync_info
                if si is not None and len(si.on_wait) > 1:
                    waits = list(si.on_wait)
                    for w in waits[:-1]:
                        _nop_ctr[0] += 1
                        nop = mybir.InstNoOp(name=f"waitnop-{_nop_ctr[0]}", ins=[], outs=[])
                        nop.engine = inst.engine
                        nop.sync_info = mybir.SyncInfo(on_wait=[w], on_update=[])
                        out.append(nop)
                    inst.sync_info = mybir.SyncInfo(
                        on_wait=[waits[-1]], on_update=list(si.on_update))
                    changed = True
                out.append(inst)
            if changed:
                b.instructions = out

_orig_to_json_bytes = bass.Bass.to_json_bytes

def _patched_to_json_bytes(self):
    if not getattr(self, "_isa_subclasses_lowered", False):
        mybir.codegen_inst_isa_subclasses(self)
        self._isa_subclasses_lowered = True
    _split_multi_waits(self.m)
    return _orig_to_json_bytes(self)

bass.Bass.to_json_bytes = _patched_to_json_bytes


# ---- kernel library ----
"""MaxK-GIN conv kernel v3 for trn2, 8-core SPMD — bf16 data path.

Structure: phase 1 sparsifies the local feat shard (top-32/row via DVE
max+match_replace), phase 2 AllGathers the bf16 sparse table, phase 3
gathers per-edge source rows (SWDGE dma_gather over 4 queues — the
bottleneck resource: ~27GB/s per queue) and scatter-adds them into
per-dst-block PSUM via selection matmuls, then applies the GIN residual
and 2-layer MLP.

v3 tuning (hardware-measured, ~404us/call vs 654us for v2):
- W=128 ladder windows: fewest selection groups (813 vs 929), fewer
  gather descriptors — descriptor count is what gathers pay for.
- 10 rotating message-tile slots keep all 4 SWDGE queues draining.
- sel matrices + bf16 feat residual streamed per block (SBUF headroom);
  idx resident.
"""
import numpy as np
import ml_dtypes

import concourse.bass as bass
import concourse.mybir as mybir
import concourse.tile as tile
from concourse import library_config

F32 = mybir.dt.float32
BF16 = mybir.dt.bfloat16
I16 = mybir.dt.int16
I32 = mybir.dt.int32

D = 128
MAXK = 32
W = 128         # selection window width (columns)
BLK = 512       # dst columns per PSUM block
NEG = -3.0e38   # match_replace fill
BF = ml_dtypes.bfloat16


# ---------------------------------------------------------------- host prep
def host_prep(feat, W1, b1, W2, b2, eps, edge_src, edge_dst, M,
              loc_split=False):
    N = feat.shape[0]
    NS = N // M
    NB = (NS + BLK - 1) // BLK
    HALF = (N // 2 + 127) & ~127  # table split point for int16 gather indices
    assert HALF <= 32768 and N - HALF <= 32768

    feat = np.asarray(feat, np.float32)
    edge_src = np.asarray(edge_src, np.int64)
    edge_dst = np.asarray(edge_dst, np.int64)
    eps_v = float(np.asarray(eps).reshape(-1)[0])

    per_core = []
    for c in range(M):
        lo = c * NS
        m = (edge_dst >= lo) & (edge_dst < lo + NS)
        ed = edge_dst[m] - lo
        es = edge_src[m]
        order = np.argsort(ed, kind="stable")
        per_core.append((ed[order], es[order]))

    def ladder_offsets(span, g):
        if g <= 0:
            return []
        if g == 1:
            return [0]
        lim = max(0, min(span, BLK) - W)
        step = lim / (g - 1) if g > 1 else 0
        return [min(int(round(j * step)), lim) for j in range(g)]

    def ladder_pack(cols, offs):
        g = len(offs)
        cnt = [0] * g
        j = 0
        for c in cols:
            while j < g and (c >= offs[j] + W or cnt[j] >= 128):
                j += 1
            if j >= g or c < offs[j]:
                return None
            cnt[j] += 1
        return cnt

    def min_groups(cols, span):
        if len(cols) == 0:
            return 0
        g = max(1, (len(cols) + 127) // 128)
        while ladder_pack(cols, ladder_offsets(span, g)) is None:
            g += 1
        return g

    NH = 3  # categories per block: 0=local shard, 1=remote-lo, 2=remote-hi
    groups = [[[None] * NH for _ in range(NB)] for _ in range(M)]
    G = np.zeros((NB, NH), np.int64)
    for c in range(M):
        ed, es = per_core[c]
        slo, shi = c * NS, (c + 1) * NS
        for b in range(NB):
            blo = b * BLK
            bw = min(BLK, NS - blo)
            bm = (ed >= blo) & (ed < blo + bw)
            bcols = ed[bm] - blo
            bsrc = es[bm]
            loc = ((bsrc >= slo) & (bsrc < shi) if loc_split
                   else np.zeros(bsrc.shape, bool))
            for h in range(NH):
                if h == 0:
                    hm = loc
                elif h == 1:
                    hm = (bsrc < HALF) & ~loc
                else:
                    hm = (bsrc >= HALF) & ~loc
                hc = bsrc[hm]
                hcols = bcols[hm]
                o = np.argsort(hcols, kind="stable")
                groups[c][b][h] = (hcols[o], hc[o])
                G[b, h] = max(G[b, h], min_groups(np.sort(hcols), bw))

    for b in range(NB):
        bw = min(BLK, NS - b * BLK)
        for h in range(NH):
            ok = False
            while not ok:
                ok = True
                offs = ladder_offsets(bw, int(G[b, h]))
                for c in range(M):
                    if ladder_pack(np.sort(groups[c][b][h][0]), offs) is None:
                        G[b, h] += 1
                        ok = False
                        break

    G_tot = int(G.sum())
    blk_cols = [(int(G[b].sum()) * 128) // 16 for b in range(NB)]
    tot_cols = sum(blk_cols)

    in_maps = []
    for c in range(M):
        idx_all = np.zeros((128, tot_cols), np.int16)
        sel_all = np.zeros((128, G_tot * W), BF)
        col0 = 0
        gflat = 0
        for b in range(NB):
            bw = min(BLK, NS - b * BLK)
            flat_idx = np.zeros((int(G[b].sum()) * 128,), np.int16)
            gi = 0
            for h in range(NH):
                gh = int(G[b, h])
                offs = ladder_offsets(bw, gh)
                hcols, hsrcs = groups[c][b][h]
                cnt = ladder_pack(hcols, offs)
                assert cnt is not None
                base = c * NS if h == 0 else (0 if h == 1 else HALF)
                p = 0
                for k in range(gh):
                    nmsg = cnt[k]
                    gcols = hcols[p:p + nmsg]
                    gsrcs = hsrcs[p:p + nmsg]
                    p += nmsg
                    flat_idx[gi * 128: gi * 128 + nmsg] = (gsrcs - base).astype(np.int16)
                    off = offs[k]
                    rel = gcols - off
                    assert nmsg == 0 or ((rel >= 0).all() and (rel < W).all())
                    sel_all[np.arange(nmsg), gflat * W + rel] = BF(1.0)
                    gi += 1
                    gflat += 1
                assert p == len(hcols)
            wrapped = flat_idx.reshape(-1, 16).T
            nb_cols = blk_cols[b]
            for r in range(8):
                idx_all[16 * r:16 * (r + 1), col0:col0 + nb_cols] = wrapped
            col0 += nb_cols
        in_maps.append({
            "feat_shard": feat[c * NS:(c + 1) * NS],
            "idx_all": idx_all,
            "sel_all": sel_all,
            "w1t": np.ascontiguousarray(np.asarray(W1, np.float32).T).astype(BF),
            "b1c": np.asarray(b1, np.float32).reshape(128, 1),
            "w2t": np.ascontiguousarray(np.asarray(W2, np.float32).T).astype(BF),
            "b2rep": np.tile(np.asarray(b2, np.float32), (128, 4)).reshape(128, 512),
            "ident_s": (np.eye(128, dtype=np.float32) * (1.0 + eps_v)).astype(BF),
        })

    lad = []
    for b in range(NB):
        bw = min(BLK, NS - b * BLK)
        lad.append([ladder_offsets(bw, int(G[b, h])) for h in range(NH)])
    meta = dict(N=N, M=M, NS=NS, NB=NB, HALF=HALF,
                G=G, G_tot=G_tot, blk_cols=blk_cols, tot_cols=tot_cols, lad=lad)
    return meta, in_maps


# ---------------------------------------------------------------- device build
def build_nc(meta, phases=(1, 2, 3), reps=1, no_selmm=False, no_gather=False,
             msg_bufs=1, gather_cs=None, ntags=4, nqueues=4, fat=False,
             scratch=16384):
    """gather_cs: max groups per dma_gather (None = no split); ntags: number
    of rotating message-tile slots (in-flight gathers)."""
    phases = set(phases)
    N, M, NS, NB = meta["N"], meta["M"], meta["NS"], meta["NB"]
    HALF, G, G_tot = meta["HALF"], meta["G"], meta["G_tot"]
    blk_cols, tot_cols = meta["blk_cols"], meta["tot_cols"]
    Gmax_half = int(G.max())
    NT = (NS + 127) // 128          # 128-row tiles in the shard

    nc = bass.Bass(num_swdge_queues=4, dynamic_dma_scratch_size=scratch)
    feat_shard = nc.dram_tensor("feat_shard", [NS, D], F32, kind="ExternalInput")
    idx_all = nc.dram_tensor("idx_all", [128, tot_cols], I16, kind="ExternalInput")
    sel_all = nc.dram_tensor("sel_all", [128, G_tot * W], BF16, kind="ExternalInput")
    w1t = nc.dram_tensor("w1t", [D, D], BF16, kind="ExternalInput")
    b1c = nc.dram_tensor("b1c", [D, 1], F32, kind="ExternalInput")
    w2t = nc.dram_tensor("w2t", [D, D], BF16, kind="ExternalInput")
    b2rep = nc.dram_tensor("b2rep", [D, BLK], F32, kind="ExternalInput")
    ident_s = nc.dram_tensor("ident_s", [D, D], BF16, kind="ExternalInput")
    out = nc.dram_tensor("out", [NS, D], F32, kind="ExternalOutput")

    shard_fs = nc.dram_tensor("shard_fs", [NS, D], BF16)
    shard_fbf = nc.dram_tensor("shard_fbf", [NS, D], BF16)
    table = nc.dram_tensor("table", [N + (128 if fat else 0), D], BF16,
                           addr_space="Shared")

    nc.gpsimd.load_library(library_config.mlp)

    reg_cache = {}

    def nreg(v):
        if v not in reg_cache:
            reg_cache[v] = nc.gpsimd.to_reg(v)
        return reg_cache[v]

    # phase-1 chunks: 4 tiles (512 rows) per DMA, last chunk is the remainder
    chunks = []
    r = 0
    while r < NS:
        nrow = min(512, NS - r)
        chunks.append((r, nrow))
        r += nrow

    with tile.TileContext(nc) as tc:
      for _rep in range(reps):
        with (
            tc.tile_pool(name=f"const{_rep}", bufs=1) as constp,
            tc.tile_pool(name=f"keep{_rep}", bufs=1) as keepp,
        ):
            # ---------------- persistent SBUF state
            w1t_sb = constp.tile([D, D], BF16)
            nc.sync.dma_start(w1t_sb[:], w1t[:, :])
            w2t_sb = constp.tile([D, D], BF16)
            nc.sync.dma_start(w2t_sb[:], w2t[:, :])
            b1_sb = constp.tile([D, 1], F32)
            nc.sync.dma_start(b1_sb[:], b1c[:, :])
            b2_sb = constp.tile([D, BLK], F32)
            nc.sync.dma_start(b2_sb[:], b2rep[:, :])
            id_sb = constp.tile([D, D], BF16)
            nc.sync.dma_start(id_sb[:], ident_s[:, :])
            zrow = constp.tile([1, D], BF16)
            nc.vector.memset(zrow[:], 0.0)
            orow = constp.tile([1, BLK], BF16)
            nc.vector.memset(orow[:], 1.0)
            if no_gather:
                dummy_src = constp.tile([128, D], BF16)
                nc.vector.memset(dummy_src[:], 0.5)

            idx_sb = keepp.tile([128, tot_cols], I16)
            nc.sync.dma_start(idx_sb[:], idx_all[:, :])

            # ---------------- phase 1: top-32 threshold + sparsify (bf16 out)
            with (
                tc.tile_pool(name=f"p1_{_rep}", bufs=3) as p1p,
                tc.tile_pool(name=f"p1w{_rep}", bufs=2) as p1w,
            ):
                for ci, (r0, nrow) in (enumerate(chunks) if 1 in phases else []):
                    nt = (nrow + 127) // 128
                    ft = p1p.tile([128, 4, D], F32, tag="ft")
                    if _rep > 0 and ci == 0:
                        # cross-rep serializer for replicated timing builds:
                        # read prev rep's output into ft, then overwrite
                        nc.sync.dma_start(ft[:, 0, :], out[0:128, :])
                    if nrow % 128 == 0:
                        nc.sync.dma_start(
                            ft[:, 0:nt, :],
                            feat_shard.ap()[r0:r0 + nrow, :]
                            .rearrange("(t p) c -> p t c", p=128))
                    else:
                        assert nt == 1
                        nc.sync.dma_start(ft[:nrow, 0, :],
                                          feat_shard[r0:r0 + nrow, :])
                    fs = p1p.tile([128, 4, D], BF16, tag="fs")
                    fbf = p1p.tile([128, 4, D], BF16, tag="fbf")
                    for j in range(nt):
                        p = min(128, nrow - j * 128)
                        m8 = p1w.tile([128, 8], F32, tag="m8")
                        wa = p1w.tile([128, D], F32, tag="wa")
                        wb = p1w.tile([128, D], F32, tag="wb")
                        nc.vector.max(m8[:p], ft[:p, j, :])
                        nc.vector.match_replace(wa[:p, :], m8[:p], ft[:p, j, :], NEG)
                        nc.vector.max(m8[:p], wa[:p, :])
                        nc.vector.match_replace(wb[:p, :], m8[:p], wa[:p, :], NEG)
                        nc.vector.max(m8[:p], wb[:p, :])
                        nc.vector.match_replace(wa[:p, :], m8[:p], wb[:p, :], NEG)
                        nc.vector.max(m8[:p], wa[:p, :])
                        mask = p1w.tile([128, D], F32, tag="mask")
                        nc.vector.tensor_scalar(mask[:p, :], ft[:p, j, :],
                                                m8[:p, 7:8], None,
                                                op0=mybir.AluOpType.is_ge)
                        nc.vector.tensor_mul(fs[:p, j, :], ft[:p, j, :], mask[:p, :])
                        # bf16 feat copy for the residual matmuls (via DRAM)
                        nc.scalar.copy(fbf[:p, j, :], ft[:p, j, :])
                    if nrow % 128 == 0:
                        nc.sync.dma_start(
                            shard_fs.ap()[r0:r0 + nrow, :]
                            .rearrange("(t p) c -> p t c", p=128),
                            fs[:, 0:nt, :])
                        nc.sync.dma_start(
                            shard_fbf.ap()[r0:r0 + nrow, :]
                            .rearrange("(t p) c -> p t c", p=128),
                            fbf[:, 0:nt, :])
                    else:
                        nc.sync.dma_start(shard_fs[r0:r0 + nrow, :],
                                          fs[:nrow, 0, :])
                        nc.sync.dma_start(shard_fbf[r0:r0 + nrow, :],
                                          fbf[:nrow, 0, :])

            # ---------------- phase 3: dst blocks (local gathers precede the
            # allgather on Pool so their drains overlap the collective)
            Gmax_blk = max(int(G[b].sum()) for b in range(NB))
            with (
                tc.tile_pool(name=f"msg{_rep}", bufs=msg_bufs) as msgp,
                tc.tile_pool(name=f"loc{_rep}", bufs=1) as locp,
                tc.tile_pool(name=f"mlp{_rep}", bufs=2) as mlpp,
                tc.tile_pool(name=f"sel{_rep}", bufs=2) as selp,
                tc.tile_pool(name=f"ps{_rep}", bufs=2, space="PSUM") as psp,
                tc.tile_pool(name=f"psh{_rep}", bufs=3, space="PSUM") as psph,
            ):
                qi = 0
                loc_tiles = {}
                if 3 in phases:
                    col0 = 0
                    for b in range(NB):
                        gloc = int(G[b, 0])
                        if not gloc:
                            col0 += blk_cols[b]
                            continue
                        lt = locp.tile([128, gloc, D], BF16,
                                       tag=f"loc{b}")
                        if not no_gather:
                            nc.gpsimd.dma_gather(
                                out_ap=lt[:, 0:gloc, :],
                                in_ap=shard_fs[0:NS, :],
                                idxs_ap=idx_sb[:, col0:col0 + gloc * 8],
                                num_idxs=gloc * 128,
                                num_idxs_reg=nreg(gloc * 128),
                                elem_size=D, single_packet=False,
                                queue_num=qi % nqueues)
                            qi += 1
                        loc_tiles[b] = lt
                        col0 += blk_cols[b]

                # ---------------- phase 2: allgather (bf16)
                if 2 in phases:
                    nc.gpsimd.collective_compute(
                        "AllGather", mybir.AluOpType.bypass,
                        replica_groups=[list(range(M))],
                        ins=[shard_fs.ap().opt()],
                        outs=[table[0:N, :].opt()],
                    )

                col0 = 0
                gflat = 0
                for b in (range(NB) if 3 in phases else []):
                    bw = min(BLK, NS - b * BLK)
                    gloc = int(G[b, 0])
                    glo, ghi = int(G[b, 1]), int(G[b, 2])
                    gb = gloc + glo + ghi
                    nb_cols = blk_cols[b]

                    # streamed per-block sel slice + bf16 feat slice
                    selb = selp.tile([128, Gmax_blk * W], BF16, tag="sel")
                    nc.sync.dma_start(
                        selb[:, 0:gb * W],
                        sel_all[:, gflat * W:(gflat + gb) * W])
                    fb = selp.tile([128, 4, D], BF16, tag="fb")
                    if bw % 128 == 0:
                        nc.sync.dma_start(
                            fb[:, 0:bw // 128, :],
                            shard_fbf.ap()[b * BLK:b * BLK + bw, :]
                            .rearrange("(t p) c -> p t c", p=128))
                    else:
                        nc.sync.dma_start(fb[:bw, 0, :],
                                          shard_fbf[b * BLK:b * BLK + bw, :])

                    CS = gather_cs if gather_cs else Gmax_half
                    TS = min(CS, Gmax_half)   # tile size in groups

                    def gather_half(nh, icol0, tlo, thi):
                        """Gather nh groups for one table half into chunked
                        tiles; returns list of (tile, base_group)."""
                        nonlocal qi
                        tiles = []
                        ew = 2 * D if fat else D
                        for o in range(0, nh, CS):
                            n = min(CS, nh - o)
                            md = msgp.tile([128, TS, ew], BF16,
                                           tag=f"mq{qi % ntags}")
                            if not no_gather:
                                nc.gpsimd.dma_gather(
                                    out_ap=md[:, 0:n, :],
                                    in_ap=table[tlo:thi + (128 if fat else 0), :],
                                    idxs_ap=idx_sb[:, icol0 + o * 8:
                                                   icol0 + (o + n) * 8],
                                    num_idxs=n * 128,
                                    num_idxs_reg=nreg(n * 128),
                                    elem_size=ew,
                                    elem_step=(D if fat else None),
                                    single_packet=False,
                                    queue_num=qi % nqueues)
                            qi += 1
                            tiles.append(md)
                        return tiles

                    lo_tiles = gather_half(glo, col0 + gloc * 8, 0, HALF)
                    hi_tiles = gather_half(ghi, col0 + (gloc + glo) * 8,
                                           HALF, N)

                    def src_of(g):
                        if g < gloc:
                            return loc_tiles[b][:, g, :]
                        if g < gloc + glo:
                            gl = g - gloc
                            return lo_tiles[gl // CS][:, gl % CS, 0:D]
                        gh = g - gloc - glo
                        return hi_tiles[gh // CS][:, gh % CS, 0:D]

                    hps = psph.tile([D, BLK], F32, tag="hps")
                    nc.tensor.matmul(hps[:, :], zrow[:, :], orow[:, :],
                                     start=True, stop=False)
                    if no_selmm and not no_gather:
                        # consumers so gather completion is still awaited
                        mr = mlpp.tile([128, 8], F32, tag="mr")
                        for ti, md in enumerate(lo_tiles + hi_tiles):
                            nc.vector.max(mr[:], md[:, :, 0:1])
                        if gloc:
                            nc.vector.max(mr[:], loc_tiles[b][:, :, 0:1])
                    lad = meta["lad"][b]
                    loffs = list(lad[0]) + list(lad[1]) + list(lad[2])
                    for g in (range(gb) if not no_selmm else []):
                        off = loffs[g]
                        if no_gather:
                            src = dummy_src[:, :]
                        else:
                            src = src_of(g)
                        nc.tensor.matmul(
                            hps[:, off:off + W],
                            src,
                            selb[:, g * W:(g + 1) * W],
                            start=False, stop=False)
                    # residual: += (1+eps) * feat^T from streamed bf16 copy
                    rr, t = 0, 0
                    while rr < bw:
                        p = min(128, bw - rr)
                        nc.tensor.matmul(
                            hps[:, rr:rr + p],
                            fb[:p, t, :],
                            id_sb[:p, :p],
                            start=False, stop=(rr + p >= bw))
                        rr += p
                        t += 1

                    # MLP layer 1 (h cast to bf16 on ACT)
                    h_sb = mlpp.tile([D, BLK], BF16, tag="h")
                    nc.scalar.copy(h_sb[:, :bw], hps[:, :bw])
                    y1ps = psp.tile([D, BLK], F32, tag="y1ps")
                    nc.tensor.matmul(y1ps[:, :bw], w1t_sb[:, :], h_sb[:, :bw],
                                     start=True, stop=True)
                    y1_sb = mlpp.tile([D, BLK], BF16, tag="y1")
                    nc.scalar.activation(y1_sb[:, :bw], y1ps[:, :bw],
                                         mybir.ActivationFunctionType.Relu,
                                         bias=b1_sb[:, 0:1], scale=1.0)
                    # MLP layer 2 fused with output transpose
                    y2ps = psp.tile([128, 4, D], F32, tag="y2ps")
                    o_sb = mlpp.tile([128, 4, D], F32, tag="o")
                    rr, t = 0, 0
                    while rr < bw:
                        p = min(128, bw - rr)
                        nc.tensor.matmul(
                            y2ps[:p, t, :],
                            y1_sb[:, rr:rr + p],
                            w2t_sb[:, :],
                            start=True, stop=True)
                        nc.vector.tensor_add(o_sb[:p, t, :], y2ps[:p, t, :],
                                             b2_sb[:p, t * D:(t + 1) * D])
                        rr += p
                        t += 1
                    if bw % 128 == 0:
                        nc.sync.dma_start(
                            out.ap()[b * BLK:b * BLK + bw, :]
                            .rearrange("(t p) c -> p t c", p=128),
                            o_sb[:, 0:t, :])
                    else:
                        assert t == 1
                        nc.sync.dma_start(out[b * BLK:b * BLK + bw, :],
                                          o_sb[:bw, 0, :])

                    col0 += nb_cols
                    gflat += gb
    return nc


# ---------------------------------------------------------------- entry point
def kernel(**inputs):
    from concourse.bass_utils import run_bass_kernel_spmd

    M = 8
    feat = np.asarray(inputs["feat"], np.float32)
    meta, in_maps = host_prep(
        feat, inputs["W1"], inputs["b1"], inputs["W2"], inputs["b2"],
        inputs["eps"], inputs["edge_src"], inputs["edge_dst"], M)
    nc = build_nc(meta, ntags=12)
    res = run_bass_kernel_spmd(nc, in_maps, core_ids=list(range(M)))
    out = np.concatenate([res.results[c]["out"] for c in range(M)], axis=0)
    return out.astype(np.float32)

